# revision 5
# baseline (speedup 1.0000x reference)
"""Causal self-attention (B=1, T=4096, C=1024, 16 heads x 64) on 8 TRN2 cores.

Sharding: tensor-parallel over heads. Core i computes heads (2i, 2i+1):
its slice of qkv, full causal attention for those heads, and the partial
output projection over its 128 y-dims. Host sums the 8 partial outputs.

v2: all matmul inputs in bf16 (exp/copies produce bf16), attention AV
flipped to produce y[q, d] directly via 65-col matmuls (64 V dims + ones
column for the softmax denominator), halving AV PE time and removing the
V-transpose (V is computed key-major straight from x). Softmax division
is folded into the PSUM->SBUF stage copy as a per-partition scalar mul.
PSUM (8 banks): scores 2x[128,1024](4), y 2x[128,512](2), aux ring (2).
Per-band y accumulators pack 4 query-subchunks x 65 cols into one bank:
start=True only on the band's first matmul into the bank (lazy zero-region
clear covers the rest), stop=True only on the last.
"""
import sys

sys.path.insert(0, "/opt/trn_rl_repo")

from contextlib import ExitStack

import numpy as np
import ml_dtypes

import concourse.bacc as bacc
import concourse.mybir as mybir
import concourse.tile as tile
from concourse.bass_utils import run_bass_kernel_spmd

F32 = mybir.dt.float32
BF16 = mybir.dt.bfloat16
EXP = mybir.ActivationFunctionType.Exp

P = 128
T = 4096
C = 1024
NH = 16
D = 64
NCORES = 8
HPC = NH // NCORES          # heads per core = 2
TB = 512                    # q-band width
NB = T // TB                # 8 bands
NKC = T // P                # 32 k-chunks
NCC = C // P                # 8 c-chunks
VW = D + 1                  # V block stride: 64 dims + ones col
PRE_N = 3                   # next-band score chunks precomputed at band end
SCALE = 1.0 / (D ** 0.5)

_cache = {}


def _build():
    nc = bacc.Bacc("TRN2", target_bir_lowering=False, debug=False,
                   num_devices=NCORES)
    xT_d = nc.dram_tensor("xT", [C, T], BF16, kind="ExternalInput").ap()
    wqkv_d = nc.dram_tensor("wqkv", [C, 3 * P], BF16, kind="ExternalInput").ap()
    wp_d = nc.dram_tensor("wp", [P, C], BF16, kind="ExternalInput").ap()
    aux_d = nc.dram_tensor("aux", [P, P + NKC], BF16, kind="ExternalInput").ap()
    ident_d = nc.dram_tensor("ident", [P, P], F32, kind="ExternalInput").ap()
    out_d = nc.dram_tensor("out", [T, C], F32, kind="ExternalOutput").ap()

    out_d3 = out_d.rearrange("(n p) c -> p n c", p=P)   # n = 128-row block
    xT3 = xT_d.rearrange("(cc p) t -> p cc t", p=P)

    with tile.TileContext(nc) as tc:
        with ExitStack() as ctx:
            const = ctx.enter_context(tc.tile_pool(name="const", bufs=1))
            big = ctx.enter_context(tc.tile_pool(name="big", bufs=1))

            auxm = const.tile([P, P + NKC], BF16)     # [tri | ones]
            nc.sync.dma_start(auxm[:], aux_d[:])
            ident = const.tile([P, P], F32)
            nc.sync.dma_start(ident[:], ident_d[:])
            wqkv = const.tile([P, NCC * 3 * P], BF16)   # [p, cc * 384]
            for cc in range(NCC):
                nc.sync.dma_start(
                    wqkv[:, cc * 3 * P:(cc + 1) * 3 * P],
                    wqkv_d[cc * P:(cc + 1) * P, :])
            wp = const.tile([P, C], BF16)
            nc.sync.dma_start(wp[:], wp_d[:])

            # full x in SBUF, one DMA per band slice: [p, cc, t-band]
            xf = big.tile([P, NCC * T], BF16)
            xf3 = xf[:].rearrange("p (cc t) -> p cc t", cc=NCC)
            for b in range(NB):
                nc.sync.dma_start(xf3[:, :, b * TB:(b + 1) * TB],
                                  xT3[:, :, b * TB:(b + 1) * TB])

            qt = big.tile([P, T], BF16)     # Q^T rows 0-63 h0, 64-127 h1
            kt = big.tile([P, T], BF16)
            von = [big.tile([P, NKC * VW], BF16, name=f"von{h}")
                   for h in range(HPC)]
            yt = big.tile([P, T], BF16)     # normalized y^T (d-major)

            # ones columns at 64::65 (softmax denominators via AV matmul)
            for h in range(HPC):
                nc.vector.tensor_copy(
                    von[h][:, D:D + (NKC - 1) * VW + 1:VW],
                    auxm[:, P:P + NKC])

            with ExitStack() as c1:
                sexp_pool = c1.enter_context(tc.tile_pool(name="sexp", bufs=8))
                rpool = c1.enter_context(tc.tile_pool(name="rc", bufs=2))
                ysb_pool = c1.enter_context(tc.tile_pool(name="ysb", bufs=6))
                opool = c1.enter_context(tc.tile_pool(name="osb", bufs=2))
                ps_s = c1.enter_context(
                    tc.tile_pool(name="ps_s", bufs=2, space="PSUM"))
                ps_y = c1.enter_context(
                    tc.tile_pool(name="ps_y", bufs=1, space="PSUM"))
                ps_a = c1.enter_context(
                    tc.tile_pool(name="ps_a", bufs=2, space="PSUM"))

                # ---------- qkv building blocks ----------
                def qk_unit(tb, mt, dest):
                    """q or k for band tb: [128, TB] accumulated over 8 cc."""
                    ps = ps_a.tile([P, TB], F32, name="aux")
                    for cc in range(NCC):
                        nc.tensor.matmul(
                            ps[:],
                            wqkv[:, cc * 3 * P + mt * P:cc * 3 * P + (mt + 1) * P],
                            xf3[:, cc, tb * TB:(tb + 1) * TB],
                            start=(cc == 0), stop=(cc == NCC - 1))
                    with nc.allow_low_precision(reason="bf16 qk"):
                        nc.vector.tensor_copy(
                            dest[:, tb * TB:(tb + 1) * TB], ps[:])

                def v_unit(tb, j):
                    """V key-chunk kc=4*tb+j, key-major [128 t, 128 dv]."""
                    kc = 4 * tb + j
                    ps = ps_a.tile([P, TB], F32, name="aux")
                    for cc in range(NCC):
                        nc.tensor.matmul(
                            ps[:, 0:P],
                            xf3[:, cc, kc * P:(kc + 1) * P],
                            wqkv[:, cc * 3 * P + 2 * P:(cc + 1) * 3 * P],
                            start=(cc == 0), stop=(cc == NCC - 1))
                    with nc.allow_low_precision(reason="bf16 v"):
                        for h in range(HPC):
                            nc.gpsimd.tensor_copy(
                                von[h][:, kc * VW:kc * VW + D],
                                ps[:, h * D:(h + 1) * D])

                # ---------- scores + exp (+ diag mask) ----------
                def scores_exp(tb, kc):
                    j = kc - 4 * tb
                    col0 = j * P if j >= 0 else 0
                    s_ps = ps_s.tile([P, 2 * TB], F32, name="sm")
                    for h in range(HPC):
                        nc.tensor.matmul(
                            s_ps[:, h * TB + col0:(h + 1) * TB],
                            kt[h * D:(h + 1) * D, kc * P:(kc + 1) * P],
                            qt[h * D:(h + 1) * D,
                               tb * TB + col0:(tb + 1) * TB],
                            start=True, stop=True)
                    s_exp = sexp_pool.tile([P, 2 * TB], BF16, name="se")
                    if col0 == 0:
                        nc.scalar.activation(s_exp[:], s_ps[:], EXP,
                                             scale=SCALE)
                    else:
                        se3 = s_exp[:].rearrange("p (h q) -> p h q", h=HPC)
                        sp3 = s_ps[:].rearrange("p (h q) -> p h q", h=HPC)
                        nc.scalar.activation(se3[:, :, col0:TB],
                                             sp3[:, :, col0:TB], EXP,
                                             scale=SCALE)
                    if j >= 0:
                        for h in range(HPC):
                            nc.vector.tensor_mul(
                                s_exp[:, h * TB + col0:h * TB + col0 + P],
                                s_exp[:, h * TB + col0:h * TB + col0 + P],
                                auxm[:, 0:P])
                    return s_exp

                # ---------- per-band state ----------
                pre = {}
                tails = {}

                def make_tail_tasks(tb, ysbs):
                    """Transpose/proj/store for finished band tb. The y
                    stage copies already ran at band end (they must precede
                    the next band's writes into the shared y banks)."""
                    tasks = []

                    def tr_task(qq):
                        def run():
                            trp = ps_a.tile([P, TB], F32, name="aux")
                            nc.tensor.transpose(trp[:, 0:P], ysbs[qq][:],
                                                ident[:])
                            with nc.allow_low_precision(reason="bf16 yT"):
                                nc.vector.tensor_copy(
                                    yt[:, (4 * tb + qq) * P:
                                       (4 * tb + qq + 1) * P],
                                    trp[:, 0:P])
                        return run

                    osb = {}

                    def po_task(qq, half):
                        def run():
                            if qq % 2 == 0 and half == 0:
                                osb[qq // 2] = opool.tile([P, 2 * C], F32,
                                                          name="osb")
                            po = ps_a.tile([P, TB], F32, name="aux")
                            nc.tensor.matmul(
                                po[:], yt[:, (4 * tb + qq) * P:
                                           (4 * tb + qq + 1) * P],
                                wp[:, half * TB:(half + 1) * TB],
                                start=True, stop=True)
                            nc.gpsimd.tensor_copy(
                                osb[qq // 2][:, (qq % 2) * C + half * TB:
                                             (qq % 2) * C + (half + 1) * TB],
                                po[:])
                        return run

                    def dma_task(hb):
                        def run():
                            nc.sync.dma_start(
                                out_d3[:, 4 * tb + 2 * hb:4 * tb + 2 * hb + 2],
                                osb[hb][:].rearrange("p (j o) -> p j o", j=2))
                        return run

                    for qq in range(4):
                        tasks.append(tr_task(qq))
                    for qq in range(4):
                        tasks.append(po_task(qq, 0))
                        tasks.append(po_task(qq, 1))
                        if qq % 2 == 1:
                            tasks.append(dma_task(qq // 2))
                    return tasks

                # ---------- band loop ----------
                def band(tb):
                    nkc = 4 * tb + 4
                    # aux task list: previous band's tail + qkv for tb+2,
                    # interleaved to avoid long PE lumps
                    tasks = []
                    qkv_tasks = []
                    if tb + 2 < NB:
                        t2 = tb + 2
                        qkv_tasks = [
                            lambda t2=t2: qk_unit(t2, 0, qt),
                            lambda t2=t2: qk_unit(t2, 1, kt),
                        ] + [lambda t2=t2, j=j: v_unit(t2, j)
                             for j in range(4)]
                    tail_tasks = tails.pop(tb - 1, [])
                    # round-robin merge, tails first (they unblock DMA out)
                    while tail_tasks or qkv_tasks:
                        if tail_tasks:
                            tasks.append(tail_tasks.pop(0))
                        if tail_tasks:
                            tasks.append(tail_tasks.pop(0))
                        if qkv_tasks:
                            tasks.append(qkv_tasks.pop(0))

                    y_ps = [ps_y.tile([P, TB], F32, name=f"y{h}")
                            for h in range(HPC)]
                    for kc in range(nkc):
                        if (tb, kc) in pre:
                            s_exp = pre.pop((tb, kc))
                        else:
                            s_exp = scores_exp(tb, kc)
                        if kc == nkc - 1 and tb + 1 < NB:
                            for kc2 in range(min(PRE_N, 1)):
                                pre[(tb + 1, kc2)] = scores_exp(tb + 1, kc2)
                        j = kc - 4 * tb
                        qq0 = max(0, j)
                        for h in range(HPC):
                            vsl = von[h][:, kc * VW:(kc + 1) * VW]
                            for qq in range(qq0, 4):
                                nc.tensor.matmul(
                                    y_ps[h][:, qq * VW:(qq + 1) * VW],
                                    s_exp[:, h * TB + qq * P:
                                          h * TB + (qq + 1) * P],
                                    vsl,
                                    start=(kc == 0 and qq == 0),
                                    stop=(kc == nkc - 1 and qq == 3))
                        if kc == nkc - 1 and tb + 1 < NB:
                            for kc2 in range(1, PRE_N):
                                pre[(tb + 1, kc2)] = scores_exp(tb + 1, kc2)
                        # dispatch aux tasks, evenly over remaining slots
                        slots_left = nkc - kc
                        n_pop = -(-len(tasks) // slots_left)
                        for _ in range(n_pop):
                            tasks.pop(0)()

                    # denominators -> reciprocal row [128, 8]
                    r = rpool.tile([P, 2 * 4], F32, name="rc")
                    with nc.allow_low_precision(reason="recip"):
                        for h in range(HPC):
                            nc.vector.reciprocal(
                                r[:, 4 * h:4 * h + 4],
                                y_ps[h][:, D:3 * VW + D + 1:VW])
                    # normalize y into SBUF now: frees the shared y banks
                    # before the next band's AV starts accumulating
                    ysbs = []
                    for qq in range(4):
                        ysb = ysb_pool.tile([P, P], F32, name="ysb")
                        with nc.allow_low_precision(reason="y scale"):
                            for h in range(HPC):
                                nc.gpsimd.tensor_scalar_mul(
                                    ysb[:, h * D:(h + 1) * D],
                                    y_ps[h][:, qq * VW:qq * VW + D],
                                    r[:, 4 * h + qq:4 * h + qq + 1])
                        ysbs.append(ysb)
                    tails[tb] = make_tail_tasks(tb, ysbs)

                # ---------- prologue: qkv for bands 0 and 1 ----------
                for b in range(2):
                    qk_unit(b, 0, qt)
                    qk_unit(b, 1, kt)
                    for j in range(4):
                        v_unit(b, j)

                for tb in range(NB):
                    band(tb)
                # final band's tail runs inline
                for t in tails.pop(NB - 1):
                    t()

    nc.finalize()
    return nc


def _prep_inputs(x, w_attn, w_proj):
    bf = ml_dtypes.bfloat16
    xT = np.ascontiguousarray(x.reshape(T, C).T).astype(bf)     # [C, T]
    tri_m = (np.arange(P)[:, None] <= np.arange(P)[None, :]).astype(np.float32)
    aux = np.concatenate(
        [tri_m, np.ones((P, NKC), np.float32)], axis=1).astype(bf)
    ident = np.eye(P, dtype=np.float32)
    in_maps = []
    for i in range(NCORES):
        hs = [HPC * i + j for j in range(HPC)]
        rows = []
        for base in (0, C, 2 * C):                   # q, k, v row blocks
            for h in hs:
                rows.append(w_attn[base + h * D:base + (h + 1) * D, :])
        wqkv = np.ascontiguousarray(
            np.concatenate(rows, axis=0).T).astype(bf)           # [C, 384]
        cols = np.concatenate([np.arange(h * D, (h + 1) * D) for h in hs])
        wp = np.ascontiguousarray(w_proj[:, cols].T).astype(bf)  # [128, C]
        in_maps.append({"xT": xT, "wqkv": wqkv, "wp": wp,
                        "aux": aux, "ident": ident})
    return in_maps


def kernel(x, w_attn, w_proj):
    x = np.asarray(x, dtype=np.float32)
    w_attn = np.asarray(w_attn, dtype=np.float32)
    w_proj = np.asarray(w_proj, dtype=np.float32)
    if "nc" not in _cache:
        _cache["nc"] = _build()
    nc = _cache["nc"]
    in_maps = _prep_inputs(x, w_attn, w_proj)
    res = run_bass_kernel_spmd(nc, in_maps, core_ids=list(range(NCORES)))
    out = np.zeros((T, C), np.float64)
    for i in range(NCORES):
        out += np.asarray(res.results[i]["out"]).astype(np.float64)
    return out.astype(np.float32).reshape(1, T, C)


# revision 8
# speedup vs baseline: 1.0373x; 1.0373x over previous
"""Causal self-attention (B=1, T=4096, C=1024, 16 heads x 64) on 8 TRN2 cores.

Sharding: tensor-parallel over heads. Core i computes heads (2i, 2i+1):
its slice of qkv, full causal attention for those heads, and the partial
output projection over its 128 y-dims. Host sums the 8 partial outputs.

v3: all matmul inputs bf16; AV flipped to produce y[q, d] via 65-col
matmuls (64 V dims + ones column giving the softmax denominator), halving
AV PE cycles; V computed key-major straight from x (no V transpose); the
softmax division folds into the y PSUM->SBUF stage copy (per-partition
scalar). The kc loop is software-pipelined: AV runs two chunks behind
scores so its matmuls are always ready when PE's sequencer reaches them
(4-deep wait queue would otherwise head-of-line block). PSUM: scores
2x[128,1024](4 banks), y 2x[128,512](2), aux ring (2). y packs 4 query
subchunks x 65 cols per bank: start=True only on the band's first matmul
into the bank (zero-region clear is lazy per byte), stop=True on the last.
"""
import sys

sys.path.insert(0, "/opt/trn_rl_repo")

from contextlib import ExitStack

import numpy as np
import ml_dtypes

import concourse.bacc as bacc
import concourse.mybir as mybir
import concourse.tile as tile
from concourse.bass_utils import run_bass_kernel_spmd

F32 = mybir.dt.float32
BF16 = mybir.dt.bfloat16
EXP = mybir.ActivationFunctionType.Exp

P = 128
T = 4096
C = 1024
NH = 16
D = 64
NCORES = 8
HPC = NH // NCORES          # heads per core = 2
TB = 512                    # q-band width
NB = T // TB                # 8 bands
NKC = T // P                # 32 k-chunks
NCC = C // P                # 8 c-chunks
VW = D + 1                  # V block stride: 64 dims + ones col
PRE_N = 3                   # next-band score chunks precomputed at band end
SCALE = 1.0 / (D ** 0.5)

_cache = {}


def _build():
    nc = bacc.Bacc("TRN2", target_bir_lowering=False, debug=False,
                   num_devices=NCORES)
    xT_d = nc.dram_tensor("xT", [C, T], BF16, kind="ExternalInput").ap()
    wqkv_d = nc.dram_tensor("wqkv", [C, 3 * P], BF16, kind="ExternalInput").ap()
    wp_d = nc.dram_tensor("wp", [P, C], BF16, kind="ExternalInput").ap()
    aux_d = nc.dram_tensor("aux", [P, P + NKC], BF16, kind="ExternalInput").ap()
    ident_d = nc.dram_tensor("ident", [P, P], F32, kind="ExternalInput").ap()
    out_d = nc.dram_tensor("out", [T, C], F32, kind="ExternalOutput").ap()

    out_d3 = out_d.rearrange("(n p) c -> p n c", p=P)   # n = 128-row block
    xT3 = xT_d.rearrange("(cc p) t -> p cc t", p=P)
    wq3 = wqkv_d.rearrange("(cc p) f -> p cc f", p=P)

    with tile.TileContext(nc) as tc:
        with ExitStack() as ctx:
            const = ctx.enter_context(tc.tile_pool(name="const", bufs=1))
            big = ctx.enter_context(tc.tile_pool(name="big", bufs=1))

            # DMA order: first x band 0, then weights, x band 1, misc, rest
            xf = big.tile([P, NCC * T], BF16)
            xf3 = xf[:].rearrange("p (cc t) -> p cc t", cc=NCC)

            def x_dma(b):
                nc.sync.dma_start(xf3[:, :, b * TB:(b + 1) * TB],
                                  xT3[:, :, b * TB:(b + 1) * TB])

            x_dma(0)
            wqkv = const.tile([P, NCC * 3 * P], BF16)   # [p, cc * 384]
            nc.sync.dma_start(
                wqkv[:].rearrange("p (cc f) -> p cc f", cc=NCC), wq3[:])
            x_dma(1)
            auxm = const.tile([P, P + NKC], BF16)     # [tri | ones]
            nc.sync.dma_start(auxm[:], aux_d[:])
            ident = const.tile([P, P], F32)
            nc.sync.dma_start(ident[:], ident_d[:])
            wp = const.tile([P, C], BF16)
            nc.sync.dma_start(wp[:], wp_d[:])
            for b in range(2, NB):
                x_dma(b)

            qt = big.tile([P, T], BF16)     # Q^T rows 0-63 h0, 64-127 h1
            kt = big.tile([P, T], BF16)
            von = [big.tile([P, NKC * VW], BF16, name=f"von{h}")
                   for h in range(HPC)]
            yt = big.tile([P, T], BF16)     # normalized y^T (d-major)

            # ones columns at 64::65 (softmax denominators via AV matmul)
            for h in range(HPC):
                nc.vector.tensor_copy(
                    von[h][:, D:D + (NKC - 1) * VW + 1:VW],
                    auxm[:, P:P + NKC])

            with ExitStack() as c1:
                sexp_pool = c1.enter_context(tc.tile_pool(name="sexp", bufs=8))
                rpool = c1.enter_context(tc.tile_pool(name="rc", bufs=2))
                ysb_pool = c1.enter_context(tc.tile_pool(name="ysb", bufs=6))
                opool = c1.enter_context(tc.tile_pool(name="osb", bufs=2))
                ps_s = c1.enter_context(
                    tc.tile_pool(name="ps_s", bufs=2, space="PSUM"))
                ps_y = c1.enter_context(
                    tc.tile_pool(name="ps_y", bufs=1, space="PSUM"))
                ps_a = c1.enter_context(
                    tc.tile_pool(name="ps_a", bufs=2, space="PSUM"))

                # ---------- qkv building blocks ----------
                def qk_unit(tb, mt, dest):
                    """q or k for band tb: [128, TB] accumulated over 8 cc."""
                    ps = ps_a.tile([P, TB], F32, name="aux")
                    for cc in range(NCC):
                        nc.tensor.matmul(
                            ps[:],
                            wqkv[:, cc * 3 * P + mt * P:cc * 3 * P + (mt + 1) * P],
                            xf3[:, cc, tb * TB:(tb + 1) * TB],
                            start=(cc == 0), stop=(cc == NCC - 1))
                    with nc.allow_low_precision(reason="bf16 qk"):
                        nc.vector.tensor_copy(
                            dest[:, tb * TB:(tb + 1) * TB], ps[:])

                def v_unit(tb, j):
                    """V key-chunk kc=4*tb+j, key-major [128 t, 128 dv]."""
                    kc = 4 * tb + j
                    ps = ps_a.tile([P, TB], F32, name="aux")
                    for cc in range(NCC):
                        nc.tensor.matmul(
                            ps[:, 0:P],
                            xf3[:, cc, kc * P:(kc + 1) * P],
                            wqkv[:, cc * 3 * P + 2 * P:(cc + 1) * 3 * P],
                            start=(cc == 0), stop=(cc == NCC - 1))
                    with nc.allow_low_precision(reason="bf16 v"):
                        for h in range(HPC):
                            nc.gpsimd.tensor_copy(
                                von[h][:, kc * VW:kc * VW + D],
                                ps[:, h * D:(h + 1) * D])

                # ---------- scores + exp (+ diag mask) ----------
                def scores_exp(tb, kc):
                    j = kc - 4 * tb
                    col0 = j * P if j >= 0 else 0
                    s_ps = ps_s.tile([P, 2 * TB], F32, name="sm")
                    for h in range(HPC):
                        nc.tensor.matmul(
                            s_ps[:, h * TB + col0:(h + 1) * TB],
                            kt[h * D:(h + 1) * D, kc * P:(kc + 1) * P],
                            qt[h * D:(h + 1) * D,
                               tb * TB + col0:(tb + 1) * TB],
                            start=True, stop=True)
                    s_exp = sexp_pool.tile([P, 2 * TB], BF16, name="se")
                    if col0 == 0:
                        nc.scalar.activation(s_exp[:], s_ps[:], EXP,
                                             scale=SCALE)
                    else:
                        se3 = s_exp[:].rearrange("p (h q) -> p h q", h=HPC)
                        sp3 = s_ps[:].rearrange("p (h q) -> p h q", h=HPC)
                        nc.scalar.activation(se3[:, :, col0:TB],
                                             sp3[:, :, col0:TB], EXP,
                                             scale=SCALE)
                    if j >= 0:
                        for h in range(HPC):
                            nc.vector.tensor_mul(
                                s_exp[:, h * TB + col0:h * TB + col0 + P],
                                s_exp[:, h * TB + col0:h * TB + col0 + P],
                                auxm[:, 0:P])
                    return s_exp

                pre = {}
                tails = {}

                def make_tail_tasks(tb, ysbs):
                    """Transpose/proj/store for finished band tb. The y
                    stage copies already ran at band end (they must precede
                    the next band's writes into the shared y banks)."""
                    def tr_task(qq):
                        def run():
                            trp = ps_a.tile([P, TB], F32, name="aux")
                            nc.tensor.transpose(trp[:, 0:P], ysbs[qq][:],
                                                ident[:])
                            with nc.allow_low_precision(reason="bf16 yT"):
                                nc.vector.tensor_copy(
                                    yt[:, (4 * tb + qq) * P:
                                       (4 * tb + qq + 1) * P],
                                    trp[:, 0:P])
                        return run

                    osb = {}

                    def po_task(qq, half):
                        def run():
                            if qq % 2 == 0 and half == 0:
                                osb[qq // 2] = opool.tile([P, 2 * C], F32,
                                                          name="osb")
                            po = ps_a.tile([P, TB], F32, name="aux")
                            nc.tensor.matmul(
                                po[:], yt[:, (4 * tb + qq) * P:
                                           (4 * tb + qq + 1) * P],
                                wp[:, half * TB:(half + 1) * TB],
                                start=True, stop=True)
                            nc.gpsimd.tensor_copy(
                                osb[qq // 2][:, (qq % 2) * C + half * TB:
                                             (qq % 2) * C + (half + 1) * TB],
                                po[:])
                        return run

                    def dma_task(hb):
                        def run():
                            nc.sync.dma_start(
                                out_d3[:, 4 * tb + 2 * hb:4 * tb + 2 * hb + 2],
                                osb[hb][:].rearrange("p (j o) -> p j o", j=2))
                        return run

                    return [tr_task(qq) for qq in range(4)], \
                        [[po_task(0, 0), po_task(0, 1), po_task(1, 0),
                          po_task(1, 1), dma_task(0)],
                         [po_task(2, 0), po_task(2, 1), po_task(3, 0),
                          po_task(3, 1), dma_task(1)]]

                # ---------- band ----------
                def band(tb):
                    nkc = 4 * tb + 4
                    # aux task list: interleave prev band's tail with qkv
                    # units for band tb+2
                    def qkv_tasks(t2):
                        return [lambda: qk_unit(t2, 0, qt),
                                lambda: qk_unit(t2, 1, kt)] + \
                               [lambda j=j: v_unit(t2, j) for j in range(4)]

                    qkv_u = []
                    if tb == 0:
                        qkv_u += qkv_tasks(1)
                    if tb + 2 < NB:
                        qkv_u += qkv_tasks(tb + 2)
                    tr_t, po_t = tails.pop(tb - 1, ([], []))
                    tasks = []
                    if qkv_u:
                        tasks.append(qkv_u.pop(0))      # q first: pre needs it
                    tasks += tr_t
                    while po_t or qkv_u:
                        if qkv_u:
                            tasks.append(qkv_u.pop(0))
                        if po_t:
                            tasks += po_t.pop(0)

                    y_ps = [ps_y.tile([P, TB], F32, name=f"y{h}")
                            for h in range(HPC)]
                    sx = {}

                    def av(kc):
                        s_exp = sx.pop(kc)
                        j = kc - 4 * tb
                        for h in range(HPC):
                            vsl = von[h][:, kc * VW:(kc + 1) * VW]
                            for qq in range(max(0, j), 4):
                                nc.tensor.matmul(
                                    y_ps[h][:, qq * VW:(qq + 1) * VW],
                                    s_exp[:, h * TB + qq * P:
                                          h * TB + (qq + 1) * P],
                                    vsl,
                                    start=(kc == 0 and qq == 0),
                                    stop=(kc == nkc - 1 and qq == 3))

                    for kc in range(nkc):
                        if (tb, kc) in pre:
                            sx[kc] = pre.pop((tb, kc))
                        if kc >= 2:
                            av(kc - 2)
                        # aux tasks, evenly over remaining iterations
                        n_pop = -(-len(tasks) // (nkc - kc))
                        for _ in range(n_pop):
                            tasks.pop(0)()
                        if kc not in sx:
                            sx[kc] = scores_exp(tb, kc)

                    # drain: AV of last two chunks, next band's pre chunks
                    # interleaved to keep ACT fed across the boundary
                    for i in range(PRE_N):
                        if tb + 1 < NB:
                            pre[(tb + 1, i)] = scores_exp(tb + 1, i)
                        if i < 2:
                            av(nkc - 2 + i)
                    for kc in range(nkc - 2 + min(PRE_N, 2), nkc):
                        av(kc)

                    # denominators -> reciprocals, normalize y into SBUF
                    # (frees the shared y banks before the next band's AV)
                    r = rpool.tile([P, 2 * 4], F32, name="rc")
                    with nc.allow_low_precision(reason="recip"):
                        for h in range(HPC):
                            nc.vector.reciprocal(
                                r[:, 4 * h:4 * h + 4],
                                y_ps[h][:, D:3 * VW + D + 1:VW])
                    ysbs = []
                    for qq in range(4):
                        ysb = ysb_pool.tile([P, P], F32, name="ysb")
                        with nc.allow_low_precision(reason="y scale"):
                            for h in range(HPC):
                                nc.gpsimd.tensor_scalar_mul(
                                    ysb[:, h * D:(h + 1) * D],
                                    y_ps[h][:, qq * VW:qq * VW + D],
                                    r[:, 4 * h + qq:4 * h + qq + 1])
                        ysbs.append(ysb)
                    tails[tb] = make_tail_tasks(tb, ysbs)

                # ---------- prologue: qkv for band 0 only ----------
                qk_unit(0, 0, qt)
                qk_unit(0, 1, kt)
                for j in range(4):
                    v_unit(0, j)

                for tb in range(NB):
                    band(tb)
                # final band's tail runs inline
                tr_t, po_t = tails.pop(NB - 1)
                for t in tr_t:
                    t()
                for grp in po_t:
                    for t in grp:
                        t()

    nc.finalize()
    return nc


def _prep_inputs(x, w_attn, w_proj):
    bf = ml_dtypes.bfloat16
    xT = np.ascontiguousarray(x.reshape(T, C).T).astype(bf)     # [C, T]
    tri_m = (np.arange(P)[:, None] <= np.arange(P)[None, :]).astype(np.float32)
    aux = np.concatenate(
        [tri_m, np.ones((P, NKC), np.float32)], axis=1).astype(bf)
    ident = np.eye(P, dtype=np.float32)
    in_maps = []
    for i in range(NCORES):
        hs = [HPC * i + j for j in range(HPC)]
        rows = []
        for base in (0, C, 2 * C):                   # q, k, v row blocks
            for h in hs:
                rows.append(w_attn[base + h * D:base + (h + 1) * D, :])
        wqkv = np.ascontiguousarray(
            np.concatenate(rows, axis=0).T).astype(bf)           # [C, 384]
        cols = np.concatenate([np.arange(h * D, (h + 1) * D) for h in hs])
        wp = np.ascontiguousarray(w_proj[:, cols].T).astype(bf)  # [128, C]
        in_maps.append({"xT": xT, "wqkv": wqkv, "wp": wp,
                        "aux": aux, "ident": ident})
    return in_maps


def kernel(x, w_attn, w_proj):
    x = np.asarray(x, dtype=np.float32)
    w_attn = np.asarray(w_attn, dtype=np.float32)
    w_proj = np.asarray(w_proj, dtype=np.float32)
    if "nc" not in _cache:
        _cache["nc"] = _build()
    nc = _cache["nc"]
    in_maps = _prep_inputs(x, w_attn, w_proj)
    res = run_bass_kernel_spmd(nc, in_maps, core_ids=list(range(NCORES)))
    out = np.zeros((T, C), np.float64)
    for i in range(NCORES):
        out += np.asarray(res.results[i]["out"]).astype(np.float64)
    return out.astype(np.float32).reshape(1, T, C)
